# revision 26
# baseline (speedup 1.0000x reference)
"""Trainium2 Bass kernel for nn_GAT_30331059044728 (GATv2 message passing).

Self-contained: hardcodes shapes/sharding; only imports concourse from the
container install at /opt/trn_rl_repo.

Strategy (8 NeuronCores):
  * Nodes partitioned into 8 blocks of 6272 (=49*128); core d owns block d
    (dst-partitioning). Edges assigned to the core owning their dst.
  * Encoder/decoder MLPs data-parallel over nodes, computed in
    feature-on-partition layout (zero transposes in encoder).
  * Per GAT layer: each core computes xl=(h@wl+bl) for its block, AllGather
    builds the full xl table [50176,192] in DRAM; xr=(h@wr+br) stays local.
  * Edges sorted by dst, grouped into 49 windows of 128 dst slots; per-edge
    xl rows fetched with dma_gather (768B rows); int16 gather indices force
    a lo/hi half-table split at 25088. xr rows gathered from the local xr
    table (int16-safe: local ids < 6272).
  * Per 128-edge tile: s=xl+xr, m=LeakyReLU(s), logits e=sum(att*m) per
    head, p=exp(e); weighted messages p*xl scatter-added into a PSUM window
    accumulator via a onehot matmul (S built on-chip from dst ids); the
    softmax denominator is accumulated as 3 extra matmul columns
    (softmax computed without max-subtraction: out = sum(p*xl)/sum(p)).
  * Window finalize: head-mean of num/den + bias + relu -> h_next.
"""

import sys

sys.path.insert(0, "/opt/trn_rl_repo")

import numpy as np
from concourse import bacc, bass, mybir, tile

F32 = mybir.dt.float32
BF16 = mybir.dt.bfloat16
I16 = mybir.dt.int16

# problem constants
N = 50000
E = 500000
IDIM = 128
HLD = 64
ODIM = 8
H = 3
SLOPE = 0.2
NCORES = 8
NPB = 6272              # nodes per core (49*128)
NPAD = NCORES * NPB     # 50176
W = NPB // 128          # 49 windows
LO = NPAD // 2          # 25088 int16 split
TILE = 128
GCALL = 16              # tiles per dma_gather call (2048 idxs)
FDIM = H * HLD          # 192
ACC_COLS = FDIM + H     # 195
XLP = 256               # padded bf16 xl row (512B)


# ----------------------------------------------------------------------------
# host-side edge plan
# ----------------------------------------------------------------------------

def build_edge_plan(edgeIdx):
    src = np.ascontiguousarray(edgeIdx[0]).astype(np.int64)
    dst = np.ascontiguousarray(edgeIdx[1]).astype(np.int64)

    order = np.argsort(dst, kind="stable")
    s_all, d_all = src[order], dst[order]
    core_ofs = np.searchsorted(d_all, np.arange(NCORES + 1) * NPB)

    # per-core grouped edge lists keyed (phase, window)
    per_core = []
    counts = np.zeros((NCORES, W, 2), np.int64)
    for d in range(NCORES):
        s_d = s_all[core_ofs[d]:core_ofs[d + 1]]
        t_d = d_all[core_ofs[d]:core_ofs[d + 1]] - d * NPB
        w_d = t_d >> 7
        ph_d = (s_d >= LO).astype(np.int64)
        key = ph_d * W + w_d
        o2 = np.argsort(key, kind="stable")
        s_d, t_d, key = s_d[o2], t_d[o2], key[o2]
        ofs = np.searchsorted(key, np.arange(2 * W + 1))
        per_core.append((s_d, t_d, ofs))
        cnt = ofs[1:] - ofs[:-1]
        counts[d, :, 0] = cnt[:W]
        counts[d, :, 1] = cnt[W:]

    ntiles = np.maximum(np.ceil(counts.max(axis=0) / TILE).astype(np.int64), 1)
    T_lo = int(ntiles[:, 0].sum())
    T_hi = int(ntiles[:, 1].sum())
    T = T_lo + T_hi
    lo_start = np.concatenate([[0], np.cumsum(ntiles[:, 0])])
    hi_start = np.concatenate([[0], np.cumsum(ntiles[:, 1])]) + T_lo

    idx_lo = np.zeros((NCORES, T_lo * TILE), np.int64)
    idx_hi = np.zeros((NCORES, T_hi * TILE), np.int64)
    idx_xr = np.zeros((NCORES, T * TILE), np.int64)
    dstw = np.full((NCORES, T * TILE), -1.0, np.float32)
    for d in range(NCORES):
        s_d, t_d, ofs = per_core[d]
        for w in range(W):
            for ph in range(2):
                a, b = ofs[ph * W + w], ofs[ph * W + w + 1]
                n = b - a
                stream_t = lo_start[w] if ph == 0 else hi_start[w]
                sb = stream_t * TILE
                if ph == 0:
                    idx_lo[d, sb:sb + n] = s_d[a:b]
                else:
                    idx_hi[d, (sb - T_lo * TILE):(sb - T_lo * TILE) + n] = \
                        s_d[a:b] - LO
                dstw[d, sb:sb + n] = (t_d[a:b] & 127).astype(np.float32)
                idx_xr[d, sb:sb + n] = t_d[a:b]

    def wrap16(arr):  # [n] -> [128, n//16]: 16-partition wrap replicated x8
        w = arr.reshape(-1, 16).T.astype(np.int16)
        return np.ascontiguousarray(np.tile(w, (8, 1)))

    return {
        "ntiles": ntiles, "T_lo": T_lo, "T_hi": T_hi, "T": T,
        "lo_start": lo_start, "hi_start": hi_start,
        "idx_lo_w": [wrap16(idx_lo[d]) for d in range(NCORES)],
        "idx_hi_w": [wrap16(idx_hi[d]) for d in range(NCORES)],
        "idx_xr_w": [wrap16(idx_xr[d]) for d in range(NCORES)],
        "dstw_w": [np.ascontiguousarray(
            dstw[d].reshape(-1, TILE).T) for d in range(NCORES)],
    }


def plan_calls(n_tiles):
    calls, t = [], 0
    while t < n_tiles:
        n = min(GCALL, n_tiles - t)
        calls.append((t, n))
        t += n
    return calls


# ----------------------------------------------------------------------------
# device program
# ----------------------------------------------------------------------------

def build_nc(meta, debug=False, stage="full", nwin=None, nlayers=2):
    """meta: dict with ntiles/T_lo/T_hi/T/lo_start/hi_start."""
    ntiles = meta["ntiles"]
    T_lo, T_hi, T = meta["T_lo"], meta["T_hi"], meta["T"]
    lo_start, hi_start = meta["lo_start"], meta["hi_start"]

    nc = bacc.Bacc("TRN2", target_bir_lowering=False, debug=debug,
                   num_swdge_queues=2)

    # ---- I/O ----
    xT = nc.dram_tensor("xT", [IDIM, NPB], F32, kind="ExternalInput")
    ew = {}
    for item in [
        ("enc_w0", [IDIM, HLD]), ("enc_b0", [HLD, 1]),
        ("enc_w1", [HLD, HLD]), ("enc_b1", [HLD, 1]),
        ("dec_w0", [HLD, HLD]), ("dec_b0", [HLD, 1]),
        ("dec_w1", [HLD, ODIM]), ("dec_b1", [ODIM, 1]),
        ("wl0", [HLD + 1, FDIM]), ("wr0", [HLD + 1, FDIM]),
        ("wl1", [HLD + 1, FDIM]), ("wr1", [HLD + 1, FDIM]),
        ("att0", [128, FDIM], BF16), ("att1", [128, FDIM], BF16),
        ("gbias0", [128, HLD]), ("gbias1", [128, HLD]),
        ("iota_row", [128, 128], BF16), ("ident", [128, 128]),
        ("identb", [128, 128], BF16),
        ("iota_neg", [128, 1]),
    ]:
        nm, shape = item[0], item[1]
        dt = item[2] if len(item) > 2 else F32
        ew[nm] = nc.dram_tensor(nm, shape, dt, kind="ExternalInput")
    idx_lo_d = nc.dram_tensor("idx_lo", [128, T_lo * 8], I16, kind="ExternalInput")
    idx_hi_d = nc.dram_tensor("idx_hi", [128, T_hi * 8], I16, kind="ExternalInput")
    idx_xr_d = nc.dram_tensor("idx_xr", [128, T * 8], I16, kind="ExternalInput")
    dstw_d = nc.dram_tensor("dstw", [128, T], BF16, kind="ExternalInput")
    dstf_d = nc.dram_tensor("dstf", [1, T * TILE], BF16, kind="ExternalInput")
    ones1_d = nc.dram_tensor("ones1", [1, 128], BF16, kind="ExternalInput")
    out_d = nc.dram_tensor("outT", [ODIM, NPB], F32, kind="ExternalOutput")

    # internal DRAM
    xl_local = nc.dram_tensor("xl_local", [NPB, XLP], BF16)
    xl_full = nc.dram_tensor("xl_full", [NPAD, XLP], BF16, addr_space="Shared")

    lo_calls = plan_calls(T_lo)
    hi_calls = plan_calls(T_hi)

    with tile.TileContext(nc) as tc:
        with (
            tc.tile_pool(name="const", bufs=1) as constp,
            tc.tile_pool(name="tables", bufs=1) as tabp,
            tc.tile_pool(name="bigbuf", bufs=2) as bigp,
            tc.tile_pool(name="ht", bufs=1) as htp,
            tc.tile_pool(name="hnext", bufs=1) as hnp,
            tc.tile_pool(name="stage", bufs=4) as stagep,
            tc.tile_pool(name="glo", bufs=3) as glop,
            tc.tile_pool(name="ghi", bufs=3) as ghip,
            tc.tile_pool(name="dstf", bufs=3) as dstfp,
            tc.tile_pool(name="edge", bufs=3) as edgep,
            tc.tile_pool(name="fin", bufs=2) as finp,
            tc.tile_pool(name="ps", bufs=2, space="PSUM") as psp,
            tc.tile_pool(name="psacc", bufs=2, space="PSUM") as psaccp,
            tc.tile_pool(name="pssq", bufs=2, space="PSUM") as pssqp,
        ):
            # ---- persistent SBUF ----
            consts = {}
            for nm in ["enc_w0", "enc_b0", "enc_w1", "enc_b1", "dec_w0",
                       "dec_b0", "dec_w1", "dec_b1", "wl0", "wr0", "wl1",
                       "wr1", "att0", "att1", "gbias0", "gbias1",
                       "iota_row", "ident", "identb", "iota_neg"]:
                t = constp.tile(list(ew[nm].shape),
                                BF16 if nm in ("att0", "att1", "iota_row",
                                               "identb")
                                else F32, tag=nm)
                nc.sync.dma_start(t[:], ew[nm][:])
                consts[nm] = t

            xT_sb = bigp.tile([IDIM, NPB], F32, tag="big")
            nc.sync.dma_start(xT_sb[:], xT[:])
            ilo_sb = tabp.tile([128, T_lo * 8], I16, tag="ilo")
            nc.sync.dma_start(ilo_sb[:], idx_lo_d[:])
            ihi_sb = tabp.tile([128, T_hi * 8], I16, tag="ihi")
            nc.sync.dma_start(ihi_sb[:], idx_hi_d[:])
            ixr_sb = tabp.tile([128, T * 8], I16, tag="ixr")
            nc.sync.dma_start(ixr_sb[:], idx_xr_d[:])
            dstw_sb = tabp.tile([128, T], BF16, tag="dstw")
            nc.sync.dma_start(dstw_sb[:], dstw_d[:])
            ones1_sb = tabp.tile([1, 128], BF16, tag="ones1")
            nc.sync.dma_start(ones1_sb[:], ones1_d[:])
            xr_sb = tabp.tile([128, W * FDIM], BF16, tag="xr_sb")

            # ---- encoder ----
            h0T = bigp.tile([HLD, NPB], F32, tag="big")
            for t in range(W):
                ps = psp.tile([HLD, 128], F32, tag="ps")
                nc.tensor.matmul(ps[:], lhsT=consts["enc_w0"][:],
                                 rhs=xT_sb[:, t * 128:(t + 1) * 128],
                                 start=True, stop=True)
                nc.scalar.activation(h0T[:, t * 128:(t + 1) * 128], ps[:],
                                     mybir.ActivationFunctionType.Relu,
                                     bias=consts["enc_b0"][:, 0:1])
            hT = htp.tile([HLD + 1, NPB], F32, tag="hT")
            nc.vector.memset(hT[HLD:HLD + 1, :], 1.0)
            for t in range(W):
                ps = psp.tile([HLD, 128], F32, tag="ps")
                nc.tensor.matmul(ps[:], lhsT=consts["enc_w1"][:],
                                 rhs=h0T[:, t * 128:(t + 1) * 128],
                                 start=True, stop=True)
                nc.scalar.activation(hT[0:HLD, t * 128:(t + 1) * 128], ps[:],
                                     mybir.ActivationFunctionType.Relu,
                                     bias=consts["enc_b1"][:, 0:1])

            # ---- GAT layers ----
            nl = 0 if stage == "encdec" else nlayers
            for layer in range(nl):
                wl = consts["wl%d" % layer]
                wr = consts["wr%d" % layer]
                att = consts["att%d" % layer]
                gbias = consts["gbias%d" % layer]

                # build xl_local / xr_local tables in DRAM
                for t in range(W):
                    lhsT = hT[:, t * 128:(t + 1) * 128]
                    psl = psp.tile([128, FDIM], F32, tag="ps")
                    nc.tensor.matmul(psl[:], lhsT=lhsT, rhs=wl[:],
                                     start=True, stop=True)
                    st = stagep.tile([128, XLP], BF16, tag="bldst")
                    nc.scalar.activation(st[:, 0:FDIM], psl[:],
                                         mybir.ActivationFunctionType.Copy)
                    nc.sync.dma_start(xl_local[t * 128:(t + 1) * 128, 0:FDIM],
                                      st[:, 0:FDIM])
                    psr = psp.tile([128, FDIM], F32, tag="ps")
                    nc.tensor.matmul(psr[:], lhsT=lhsT, rhs=wr[:],
                                     start=True, stop=True)
                    nc.scalar.activation(xr_sb[:, t * FDIM:(t + 1) * FDIM],
                                         psr[:],
                                         mybir.ActivationFunctionType.Copy)

                # AllGather xl
                nc.gpsimd.collective_compute(
                    "AllGather", mybir.AluOpType.bypass,
                    replica_groups=[list(range(NCORES))],
                    ins=[xl_local[:]], outs=[xl_full[:]],
                )

                if stage == "build":
                    st_dbg = stagep.tile([8, 192], F32, tag="dbg")
                    nc.sync.dma_start(st_dbg[:], xl_full[1000:1008, :])
                    nc.sync.dma_start(out_d[0:8, 0:192], st_dbg[:])
                    continue
                # gather calls (emitted lazily ahead of consumption)
                lo_bufs = [None] * len(lo_calls)
                hi_bufs = [None] * len(hi_calls)

                def emit_lo(ci):
                    t0, n = lo_calls[ci]
                    b = glop.tile([128, GCALL, XLP], BF16, tag="glo")
                    nc.gpsimd.dma_gather(
                        b[:, 0:n, :], xl_full[0:LO, :],
                        ilo_sb[:, t0 * 8:(t0 + n) * 8],
                        n * TILE, n * TILE, XLP, single_packet=False,
                        queue_num=0)
                    lo_bufs[ci] = b

                def emit_hi(ci):
                    t0, n = hi_calls[ci]
                    b = ghip.tile([128, GCALL, XLP], BF16, tag="ghi")
                    nc.gpsimd.dma_gather(
                        b[:, 0:n, :], xl_full[LO:NPAD, :],
                        ihi_sb[:, t0 * 8:(t0 + n) * 8],
                        n * TILE, n * TILE, XLP, single_packet=False,
                        queue_num=1)
                    hi_bufs[ci] = b

                next_lo = [0]
                next_hi = [0]

                def get_chunk_view(stream_t, nb):
                    # [128, nb, FDIM] view of nb consecutive stream tiles
                    if stream_t < T_lo:
                        ci, off = divmod(stream_t, GCALL)
                        while next_lo[0] <= ci:
                            emit_lo(next_lo[0])
                            next_lo[0] += 1
                        return lo_bufs[ci][:, off:off + nb, 0:FDIM]
                    st = stream_t - T_lo
                    ci, off = divmod(st, GCALL)
                    while next_hi[0] <= ci:
                        emit_hi(next_hi[0])
                        next_hi[0] += 1
                    return hi_bufs[ci][:, off:off + nb, 0:FDIM]

                h_next = hnp.tile([128, W * HLD], F32, tag="hnext")

                for w in range(W if nwin is None else nwin):
                    acc = psaccp.tile([128, 256], F32, tag="acc")
                    xr_win = xr_sb[:, w * FDIM:(w + 1) * FDIM]
                    # chunk this window's tiles: consecutive stream tiles,
                    # same phase block, same gather call, <= 4 tiles
                    chunks = []
                    for base, cnt in ((lo_start[w], ntiles[w, 0]),
                                      (hi_start[w], ntiles[w, 1])):
                        t = base
                        while t < base + cnt:
                            pos = t if t < T_lo else t - T_lo
                            nb = min(base + cnt - t, 4,
                                     GCALL - pos % GCALL)
                            chunks.append((int(t), int(nb)))
                            t += nb
                    nch = len(chunks)
                    for ci_, (b0, nb) in enumerate(chunks):
                        xl_q = get_chunk_view(b0, nb)
                        # dst row for these edges -> [1, nb*128] bf16
                        dstf = dstfp.tile([1, 4 * TILE], BF16, tag="dstf")
                        nc.sync.dma_start(
                            dstf[:, 0:nb * TILE],
                            dstf_d[0:1, b0 * TILE:(b0 + nb) * TILE])
                        # broadcast dst down partitions via K=1 matmul
                        psdb = psp.tile([128, 512], F32, tag="ps")
                        nc.tensor.matmul(
                            psdb[:, 0:nb * TILE], lhsT=ones1_sb[:],
                            rhs=dstf[:, 0:nb * TILE], start=True, stop=True)
                        # ST[j, e] = relu(1 - (dst_e - j)^2)  (exact onehot)
                        sq_t = edgep.tile([128, 512], F32, tag="sq")
                        nc.scalar.activation(
                            sq_t[:, 0:nb * TILE], psdb[:, 0:nb * TILE],
                            mybir.ActivationFunctionType.Square,
                            bias=consts["iota_neg"][:, 0:1])
                        ST_q = edgep.tile([128, 4, TILE], BF16, tag="ST")
                        nc.scalar.activation(
                            ST_q[:, 0:nb, :], sq_t[:, 0:nb * TILE],
                            mybir.ActivationFunctionType.Relu,
                            bias=1.0, scale=-1.0)
                        # s = xr_exp + xl accumulated in PSUM (two matmuls)
                        squad = pssqp.tile([128, 4, 256], F32, tag="squad")
                        for t in range(nb):
                            nc.tensor.matmul(
                                squad[:, t, 0:FDIM], lhsT=ST_q[:, t, :],
                                rhs=xr_win, start=True, stop=False)
                            nc.tensor.matmul(
                                squad[:, t, 0:FDIM], lhsT=consts["identb"][:],
                                rhs=xl_q[:, t, :], start=False, stop=True)
                        # m = leaky_relu(s) -> bf16 scratch in wm_q[:, :, 0:192]
                        wm_q = edgep.tile([128, 4, 256], BF16, tag="wm")
                        nc.scalar.activation(
                            wm_q[:, 0:nb, 0:FDIM], squad[:, 0:nb, 0:FDIM],
                            mybir.ActivationFunctionType.Prelu, alpha=SLOPE)
                        # em = m * att (bf16 2x mode, in place)
                        nc.vector.tensor_tensor(
                            out=wm_q[:, 0:nb, 0:FDIM], in0=wm_q[:, 0:nb, 0:FDIM],
                            in1=att[:].rearrange("p (o f) -> p o f", o=1)
                                .broadcast_to([128, nb, FDIM]),
                            op=mybir.AluOpType.mult)
                        # e (logits) then p = exp(e): f32 for the wm mult,
                        # bf16 straight into the scatter den columns
                        pe_q = edgep.tile([128, 4, 8], F32, tag="pe")
                        nc.vector.tensor_reduce(
                            out=pe_q[:, 0:nb, 4:7],
                            in_=wm_q[:, 0:nb, 0:FDIM].rearrange(
                                "p t (h c) -> p t h c", h=H),
                            axis=mybir.AxisListType.X, op=mybir.AluOpType.add)
                        nc.scalar.activation(
                            wm_q[:, 0:nb, FDIM:FDIM + H], pe_q[:, 0:nb, 4:7],
                            mybir.ActivationFunctionType.Exp)
                        # wm = p * xl (overwrites em scratch; all bf16)
                        nc.vector.tensor_tensor(
                            out=wm_q[:, 0:nb, 0:FDIM].rearrange(
                                "p t (h c) -> p t h c", h=H),
                            in0=xl_q.rearrange("p t (h c) -> p t h c", h=H),
                            in1=wm_q[:, 0:nb, FDIM:FDIM + H].broadcast_to(
                                [128, nb, H, HLD]),
                            op=mybir.AluOpType.mult)
                        # S onehot (edge-partition layout), bf16
                        S_q = edgep.tile([128, 4, TILE], BF16, tag="Sq")
                        nc.vector.tensor_tensor(
                            out=S_q[:, 0:nb, :],
                            in0=consts["iota_row"][:].rearrange(
                                "p (o f) -> p o f", o=1)
                                .broadcast_to([128, nb, TILE]),
                            in1=dstw_sb[:, b0:b0 + nb].broadcast_to(
                                [128, nb, TILE]),
                            op=mybir.AluOpType.is_equal)
                        for t in range(nb):
                            nc.tensor.matmul(
                                acc[:, 0:ACC_COLS], lhsT=S_q[:, t, :],
                                rhs=wm_q[:, t, 0:ACC_COLS],
                                start=(ci_ == 0 and t == 0),
                                stop=(ci_ == nch - 1 and t == nb - 1))

                    # ---- finalize window (off the PSUM critical path) ----
                    accs = finp.tile([128, 200], F32, tag="accs")
                    nc.scalar.activation(accs[:, 0:ACC_COLS],
                                         acc[:, 0:ACC_COLS],
                                         mybir.ActivationFunctionType.Copy)
                    fin = finp.tile([128, 8], F32, tag="fin")
                    nc.vector.tensor_scalar(
                        out=fin[:, 0:3], in0=accs[:, 192:195],
                        scalar1=3.0, scalar2=1e-16,
                        op0=mybir.AluOpType.mult, op1=mybir.AluOpType.add)
                    nc.vector.reciprocal(fin[:, 4:7], fin[:, 0:3])
                    u_t = finp.tile([128, FDIM], F32, tag="u")
                    for h in range(H):
                        nc.vector.tensor_scalar(
                            out=u_t[:, h * HLD:(h + 1) * HLD],
                            in0=accs[:, h * HLD:(h + 1) * HLD],
                            scalar1=fin[:, 4 + h:5 + h], scalar2=None,
                            op0=mybir.AluOpType.mult)
                    v_t = finp.tile([128, HLD], F32, tag="v")
                    nc.vector.tensor_tensor(
                        out=v_t[:], in0=u_t[:, 0:HLD], in1=u_t[:, HLD:2 * HLD],
                        op=mybir.AluOpType.add)
                    v2_t = finp.tile([128, HLD], F32, tag="v2")
                    nc.vector.tensor_tensor(
                        out=v2_t[:], in0=v_t[:], in1=u_t[:, 2 * HLD:3 * HLD],
                        op=mybir.AluOpType.add)
                    v3_t = finp.tile([128, HLD], F32, tag="v3")
                    nc.vector.tensor_tensor(
                        out=v3_t[:], in0=v2_t[:], in1=gbias[:],
                        op=mybir.AluOpType.add)
                    nc.scalar.activation(
                        h_next[:, w * HLD:(w + 1) * HLD], v3_t[:],
                        mybir.ActivationFunctionType.Relu)

                # ---- transpose h_next -> hT for next phase ----
                hT = htp.tile([HLD + 1, NPB], F32, tag="hT")
                nc.vector.memset(hT[HLD:HLD + 1, :], 1.0)
                for t in range(W):
                    pst = psp.tile([HLD, 128], F32, tag="ps")
                    nc.tensor.transpose(
                        pst[:], in_=h_next[:, t * HLD:(t + 1) * HLD],
                        identity=consts["ident"][:])
                    nc.scalar.activation(
                        hT[0:HLD, t * 128:(t + 1) * 128], pst[:],
                        mybir.ActivationFunctionType.Copy)

            # ---- decoder ----
            y0T = bigp.tile([HLD, NPB], F32, tag="big")
            for t in range(W):
                ps = psp.tile([HLD, 128], F32, tag="ps")
                nc.tensor.matmul(ps[:], lhsT=consts["dec_w0"][:],
                                 rhs=hT[0:HLD, t * 128:(t + 1) * 128],
                                 start=True, stop=True)
                nc.scalar.activation(y0T[:, t * 128:(t + 1) * 128], ps[:],
                                     mybir.ActivationFunctionType.Relu,
                                     bias=consts["dec_b0"][:, 0:1])
            outT_sb = bigp.tile([ODIM, NPB], F32, tag="big")
            for t in range(W):
                ps = psp.tile([ODIM, 128], F32, tag="ps")
                nc.tensor.matmul(ps[:], lhsT=consts["dec_w1"][:],
                                 rhs=y0T[:, t * 128:(t + 1) * 128],
                                 start=True, stop=True)
                nc.scalar.activation(outT_sb[:, t * 128:(t + 1) * 128], ps[:],
                                     mybir.ActivationFunctionType.Relu,
                                     bias=consts["dec_b1"][:, 0:1])
            nc.sync.dma_start(out_d[:], outT_sb[:])

    nc.compile()
    return nc


# ----------------------------------------------------------------------------
# host orchestration
# ----------------------------------------------------------------------------

def make_in_maps(inputs, plan):
    x = np.asarray(inputs["x"], np.float32)
    xpad = np.zeros((NPAD, IDIM), np.float32)
    xpad[:N] = x

    def col(b):
        return np.ascontiguousarray(np.asarray(b, np.float32).reshape(-1, 1))

    iota_row = np.tile(np.arange(128, dtype=np.float32), (128, 1)).astype(
        __import__("ml_dtypes").bfloat16)
    ident = np.eye(128, dtype=np.float32)

    def wplus(wname, bname):
        wm = np.asarray(inputs[wname], np.float32)
        bm = np.asarray(inputs[bname], np.float32)
        return np.ascontiguousarray(np.vstack([wm, bm[None, :]]))

    shared = {
        "enc_w0": np.ascontiguousarray(np.asarray(inputs["enc_w0"], np.float32)),
        "enc_b0": col(inputs["enc_b0"]),
        "enc_w1": np.ascontiguousarray(np.asarray(inputs["enc_w1"], np.float32)),
        "enc_b1": col(inputs["enc_b1"]),
        "dec_w0": np.ascontiguousarray(np.asarray(inputs["dec_w0"], np.float32)),
        "dec_b0": col(inputs["dec_b0"]),
        "dec_w1": np.ascontiguousarray(np.asarray(inputs["dec_w1"], np.float32)),
        "dec_b1": col(inputs["dec_b1"]),
        "wl0": wplus("gat0_wl", "gat0_bl"),
        "wr0": wplus("gat0_wr", "gat0_br"),
        "wl1": wplus("gat1_wl", "gat1_bl"),
        "wr1": wplus("gat1_wr", "gat1_br"),
        "att0": np.tile(np.asarray(inputs["gat0_att"], np.float32)
                        .reshape(1, FDIM), (128, 1)).astype(
                        __import__("ml_dtypes").bfloat16),
        "att1": np.tile(np.asarray(inputs["gat1_att"], np.float32)
                        .reshape(1, FDIM), (128, 1)).astype(
                        __import__("ml_dtypes").bfloat16),
        "gbias0": np.tile(np.asarray(inputs["gat0_bias"], np.float32)
                          .reshape(1, HLD), (128, 1)),
        "gbias1": np.tile(np.asarray(inputs["gat1_bias"], np.float32)
                          .reshape(1, HLD), (128, 1)),
        "iota_row": iota_row,
        "ident": ident,
        "iota_neg": -np.arange(128, dtype=np.float32).reshape(128, 1),
        "identb": np.eye(128, dtype=np.float32).astype(
            __import__("ml_dtypes").bfloat16),
        "ones1": np.ones((1, 128), np.float16).astype(np.float32).astype(
            __import__("ml_dtypes").bfloat16).reshape(1, 128),
    }
    maps = []
    for d in range(NCORES):
        m = dict(shared)
        m["xT"] = np.ascontiguousarray(xpad[d * NPB:(d + 1) * NPB].T)
        m["idx_lo"] = plan["idx_lo_w"][d]
        m["idx_hi"] = plan["idx_hi_w"][d]
        m["idx_xr"] = plan["idx_xr_w"][d]
        m["dstw"] = plan["dstw_w"][d].astype(
            __import__("ml_dtypes").bfloat16)
        m["dstf"] = np.ascontiguousarray(
            plan["dstw_w"][d].T.reshape(1, -1)).astype(
            __import__("ml_dtypes").bfloat16)
        maps.append(m)
    return maps


def kernel(**inputs):
    from concourse.bass_utils import run_bass_kernel_spmd

    plan = build_edge_plan(np.asarray(inputs["edgeIdx"]))
    nc = build_nc(plan)
    in_maps = make_in_maps(inputs, plan)
    res = run_bass_kernel_spmd(nc, in_maps, list(range(NCORES)))
    outs = res.results
    full = np.concatenate([outs[d]["outT"].T for d in range(NCORES)], 0)
    return np.ascontiguousarray(full[:N]).astype(np.float32)


# revision 27
# speedup vs baseline: 1.2016x; 1.2016x over previous
"""Trainium2 Bass kernel for nn_GAT_30331059044728 (GATv2 message passing).

Self-contained: hardcodes shapes/sharding; only imports concourse from the
container install at /opt/trn_rl_repo.

Strategy (8 NeuronCores):
  * Nodes partitioned into 8 blocks of 6272 (=49*128); core d owns block d
    (dst-partitioning). Edges assigned to the core owning their dst.
  * Encoder/decoder MLPs data-parallel over nodes, computed in
    feature-on-partition layout (zero transposes in encoder).
  * Per GAT layer: each core computes xl=(h@wl+bl) for its block, AllGather
    builds the full xl table [50176,192] in DRAM; xr=(h@wr+br) stays local.
  * Edges sorted by dst, grouped into 49 windows of 128 dst slots; per-edge
    xl rows fetched with dma_gather (768B rows); int16 gather indices force
    a lo/hi half-table split at 25088. xr rows gathered from the local xr
    table (int16-safe: local ids < 6272).
  * Per 128-edge tile: s=xl+xr, m=LeakyReLU(s), logits e=sum(att*m) per
    head, p=exp(e); weighted messages p*xl scatter-added into a PSUM window
    accumulator via a onehot matmul (S built on-chip from dst ids); the
    softmax denominator is accumulated as 3 extra matmul columns
    (softmax computed without max-subtraction: out = sum(p*xl)/sum(p)).
  * Window finalize: head-mean of num/den + bias + relu -> h_next.
"""

import sys

sys.path.insert(0, "/opt/trn_rl_repo")

import numpy as np
from concourse import bacc, bass, mybir, tile

F32 = mybir.dt.float32
BF16 = mybir.dt.bfloat16
I16 = mybir.dt.int16

# problem constants
N = 50000
E = 500000
IDIM = 128
HLD = 64
ODIM = 8
H = 3
SLOPE = 0.2
NCORES = 8
NPB = 6272              # nodes per core (49*128)
NPAD = NCORES * NPB     # 50176
W = NPB // 128          # 49 windows
LO = NPAD // 2          # 25088 int16 split
TILE = 128
GCALL = 16              # tiles per dma_gather call (2048 idxs)
FDIM = H * HLD          # 192
ACC_COLS = FDIM + H     # 195
XLP = 256               # padded bf16 xl row (512B)


# ----------------------------------------------------------------------------
# host-side edge plan
# ----------------------------------------------------------------------------

def build_edge_plan(edgeIdx):
    src = np.ascontiguousarray(edgeIdx[0]).astype(np.int64)
    dst = np.ascontiguousarray(edgeIdx[1]).astype(np.int64)

    order = np.argsort(dst, kind="stable")
    s_all, d_all = src[order], dst[order]
    core_ofs = np.searchsorted(d_all, np.arange(NCORES + 1) * NPB)

    # per-core grouped edge lists keyed (phase, window)
    per_core = []
    counts = np.zeros((NCORES, W, 2), np.int64)
    for d in range(NCORES):
        s_d = s_all[core_ofs[d]:core_ofs[d + 1]]
        t_d = d_all[core_ofs[d]:core_ofs[d + 1]] - d * NPB
        w_d = t_d >> 7
        ph_d = (s_d >= LO).astype(np.int64)
        key = ph_d * W + w_d
        o2 = np.argsort(key, kind="stable")
        s_d, t_d, key = s_d[o2], t_d[o2], key[o2]
        ofs = np.searchsorted(key, np.arange(2 * W + 1))
        per_core.append((s_d, t_d, ofs))
        cnt = ofs[1:] - ofs[:-1]
        counts[d, :, 0] = cnt[:W]
        counts[d, :, 1] = cnt[W:]

    ntiles = np.maximum(np.ceil(counts.max(axis=0) / TILE).astype(np.int64), 1)
    T_lo = int(ntiles[:, 0].sum())
    T_hi = int(ntiles[:, 1].sum())
    T = T_lo + T_hi
    lo_start = np.concatenate([[0], np.cumsum(ntiles[:, 0])])
    hi_start = np.concatenate([[0], np.cumsum(ntiles[:, 1])]) + T_lo

    idx_lo = np.zeros((NCORES, T_lo * TILE), np.int64)
    idx_hi = np.zeros((NCORES, T_hi * TILE), np.int64)
    idx_xr = np.zeros((NCORES, T * TILE), np.int64)
    dstw = np.full((NCORES, T * TILE), -1.0, np.float32)
    for d in range(NCORES):
        s_d, t_d, ofs = per_core[d]
        for w in range(W):
            for ph in range(2):
                a, b = ofs[ph * W + w], ofs[ph * W + w + 1]
                n = b - a
                stream_t = lo_start[w] if ph == 0 else hi_start[w]
                sb = stream_t * TILE
                if ph == 0:
                    idx_lo[d, sb:sb + n] = s_d[a:b]
                else:
                    idx_hi[d, (sb - T_lo * TILE):(sb - T_lo * TILE) + n] = \
                        s_d[a:b] - LO
                dstw[d, sb:sb + n] = (t_d[a:b] & 127).astype(np.float32)
                idx_xr[d, sb:sb + n] = t_d[a:b]

    def wrap16(arr):  # [n] -> [128, n//16]: 16-partition wrap replicated x8
        w = arr.reshape(-1, 16).T.astype(np.int16)
        return np.ascontiguousarray(np.tile(w, (8, 1)))

    return {
        "ntiles": ntiles, "T_lo": T_lo, "T_hi": T_hi, "T": T,
        "lo_start": lo_start, "hi_start": hi_start,
        "idx_lo_w": [wrap16(idx_lo[d]) for d in range(NCORES)],
        "idx_hi_w": [wrap16(idx_hi[d]) for d in range(NCORES)],
        "idx_xr_w": [wrap16(idx_xr[d]) for d in range(NCORES)],
        "dstw_w": [np.ascontiguousarray(
            dstw[d].reshape(-1, TILE).T) for d in range(NCORES)],
    }


def plan_calls(n_tiles):
    calls, t = [], 0
    while t < n_tiles:
        n = min(GCALL, n_tiles - t)
        calls.append((t, n))
        t += n
    return calls


# ----------------------------------------------------------------------------
# device program
# ----------------------------------------------------------------------------

def build_nc(meta, debug=False, stage="full", nwin=None, nlayers=2):
    """meta: dict with ntiles/T_lo/T_hi/T/lo_start/hi_start."""
    ntiles = meta["ntiles"]
    T_lo, T_hi, T = meta["T_lo"], meta["T_hi"], meta["T"]
    lo_start, hi_start = meta["lo_start"], meta["hi_start"]

    nc = bacc.Bacc("TRN2", target_bir_lowering=False, debug=debug,
                   num_swdge_queues=2)

    # ---- I/O ----
    xT = nc.dram_tensor("xT", [IDIM, NPB], F32, kind="ExternalInput")
    ew = {}
    for item in [
        ("enc_w0", [IDIM, HLD]), ("enc_b0", [HLD, 1]),
        ("enc_w1", [HLD, HLD]), ("enc_b1", [HLD, 1]),
        ("dec_w0", [HLD, HLD]), ("dec_b0", [HLD, 1]),
        ("dec_w1", [HLD, ODIM]), ("dec_b1", [ODIM, 1]),
        ("wl0", [HLD + 1, FDIM]), ("wr0", [HLD + 1, FDIM]),
        ("wl1", [HLD + 1, FDIM]), ("wr1", [HLD + 1, FDIM]),
        ("att0", [128, FDIM], BF16), ("att1", [128, FDIM], BF16),
        ("gbias0", [128, HLD]), ("gbias1", [128, HLD]),
        ("iota_row", [128, 128], BF16), ("ident", [128, 128]),
        ("identb", [128, 128], BF16),
        ("iota_neg", [128, 1]),
    ]:
        nm, shape = item[0], item[1]
        dt = item[2] if len(item) > 2 else F32
        ew[nm] = nc.dram_tensor(nm, shape, dt, kind="ExternalInput")
    idx_lo_d = nc.dram_tensor("idx_lo", [128, T_lo * 8], I16, kind="ExternalInput")
    idx_hi_d = nc.dram_tensor("idx_hi", [128, T_hi * 8], I16, kind="ExternalInput")
    idx_xr_d = nc.dram_tensor("idx_xr", [128, T * 8], I16, kind="ExternalInput")
    dstw_d = nc.dram_tensor("dstw", [128, T], BF16, kind="ExternalInput")
    dstf_d = nc.dram_tensor("dstf", [1, T * TILE], BF16, kind="ExternalInput")
    ones1_d = nc.dram_tensor("ones1", [1, 128], BF16, kind="ExternalInput")
    out_d = nc.dram_tensor("outT", [ODIM, NPB], F32, kind="ExternalOutput")

    # internal DRAM
    xl_local = nc.dram_tensor("xl_local", [NPB, XLP], BF16)
    xl_full = nc.dram_tensor("xl_full", [NPAD, XLP], BF16, addr_space="Shared")

    lo_calls = plan_calls(T_lo)
    hi_calls = plan_calls(T_hi)

    with tile.TileContext(nc) as tc:
        with (
            tc.tile_pool(name="const", bufs=1) as constp,
            tc.tile_pool(name="tables", bufs=1) as tabp,
            tc.tile_pool(name="bigbuf", bufs=2) as bigp,
            tc.tile_pool(name="ht", bufs=1) as htp,
            tc.tile_pool(name="hnext", bufs=1) as hnp,
            tc.tile_pool(name="stage", bufs=4) as stagep,
            tc.tile_pool(name="glo", bufs=3) as glop,
            tc.tile_pool(name="ghi", bufs=3) as ghip,
            tc.tile_pool(name="dstf", bufs=3) as dstfp,
            tc.tile_pool(name="edge", bufs=3) as edgep,
            tc.tile_pool(name="fin", bufs=2) as finp,
            tc.tile_pool(name="ps", bufs=2, space="PSUM") as psp,
            tc.tile_pool(name="psacc", bufs=2, space="PSUM") as psaccp,
            tc.tile_pool(name="pssq", bufs=2, space="PSUM") as pssqp,
        ):
            # ---- persistent SBUF ----
            consts = {}
            for nm in ["enc_w0", "enc_b0", "enc_w1", "enc_b1", "dec_w0",
                       "dec_b0", "dec_w1", "dec_b1", "wl0", "wr0", "wl1",
                       "wr1", "att0", "att1", "gbias0", "gbias1",
                       "iota_row", "ident", "identb", "iota_neg"]:
                t = constp.tile(list(ew[nm].shape),
                                BF16 if nm in ("att0", "att1", "iota_row",
                                               "identb")
                                else F32, tag=nm)
                nc.sync.dma_start(t[:], ew[nm][:])
                consts[nm] = t

            xT_sb = bigp.tile([IDIM, NPB], F32, tag="big")
            nc.sync.dma_start(xT_sb[:], xT[:])
            ilo_sb = tabp.tile([128, T_lo * 8], I16, tag="ilo")
            nc.sync.dma_start(ilo_sb[:], idx_lo_d[:])
            ihi_sb = tabp.tile([128, T_hi * 8], I16, tag="ihi")
            nc.sync.dma_start(ihi_sb[:], idx_hi_d[:])
            ixr_sb = tabp.tile([128, T * 8], I16, tag="ixr")
            nc.sync.dma_start(ixr_sb[:], idx_xr_d[:])
            dstw_sb = tabp.tile([128, T], BF16, tag="dstw")
            nc.sync.dma_start(dstw_sb[:], dstw_d[:])
            ones1_sb = tabp.tile([1, 128], BF16, tag="ones1")
            nc.sync.dma_start(ones1_sb[:], ones1_d[:])
            xr_sb = tabp.tile([128, W * FDIM], BF16, tag="xr_sb")

            # ---- encoder ----
            h0T = bigp.tile([HLD, NPB], F32, tag="big")
            for t in range(W):
                ps = psp.tile([HLD, 128], F32, tag="ps")
                nc.tensor.matmul(ps[:], lhsT=consts["enc_w0"][:],
                                 rhs=xT_sb[:, t * 128:(t + 1) * 128],
                                 start=True, stop=True)
                nc.scalar.activation(h0T[:, t * 128:(t + 1) * 128], ps[:],
                                     mybir.ActivationFunctionType.Relu,
                                     bias=consts["enc_b0"][:, 0:1])
            hT = htp.tile([HLD + 1, NPB], F32, tag="hT")
            nc.vector.memset(hT[HLD:HLD + 1, :], 1.0)
            for t in range(W):
                ps = psp.tile([HLD, 128], F32, tag="ps")
                nc.tensor.matmul(ps[:], lhsT=consts["enc_w1"][:],
                                 rhs=h0T[:, t * 128:(t + 1) * 128],
                                 start=True, stop=True)
                nc.scalar.activation(hT[0:HLD, t * 128:(t + 1) * 128], ps[:],
                                     mybir.ActivationFunctionType.Relu,
                                     bias=consts["enc_b1"][:, 0:1])

            # ---- GAT layers ----
            nl = 0 if stage == "encdec" else nlayers
            for layer in range(nl):
                wl = consts["wl%d" % layer]
                wr = consts["wr%d" % layer]
                att = consts["att%d" % layer]
                gbias = consts["gbias%d" % layer]

                # build xl_local / xr_local tables in DRAM
                for t in range(W):
                    lhsT = hT[:, t * 128:(t + 1) * 128]
                    psl = psp.tile([128, FDIM], F32, tag="ps")
                    nc.tensor.matmul(psl[:], lhsT=lhsT, rhs=wl[:],
                                     start=True, stop=True)
                    st = stagep.tile([128, XLP], BF16, tag="bldst")
                    nc.scalar.activation(st[:, 0:FDIM], psl[:],
                                         mybir.ActivationFunctionType.Copy)
                    nc.sync.dma_start(xl_local[t * 128:(t + 1) * 128, 0:FDIM],
                                      st[:, 0:FDIM])
                    psr = psp.tile([128, FDIM], F32, tag="ps")
                    nc.tensor.matmul(psr[:], lhsT=lhsT, rhs=wr[:],
                                     start=True, stop=True)
                    nc.scalar.activation(xr_sb[:, t * FDIM:(t + 1) * FDIM],
                                         psr[:],
                                         mybir.ActivationFunctionType.Copy)

                # AllGather xl
                nc.gpsimd.collective_compute(
                    "AllGather", mybir.AluOpType.bypass,
                    replica_groups=[list(range(NCORES))],
                    ins=[xl_local[:]], outs=[xl_full[:]],
                )

                if stage == "build":
                    st_dbg = stagep.tile([8, 192], F32, tag="dbg")
                    nc.sync.dma_start(st_dbg[:], xl_full[1000:1008, :])
                    nc.sync.dma_start(out_d[0:8, 0:192], st_dbg[:])
                    continue
                # gather calls (emitted lazily ahead of consumption)
                lo_bufs = [None] * len(lo_calls)
                hi_bufs = [None] * len(hi_calls)

                def emit_lo(ci):
                    t0, n = lo_calls[ci]
                    b = glop.tile([128, GCALL, XLP], BF16, tag="glo")
                    nc.gpsimd.dma_gather(
                        b[:, 0:n, :], xl_full[0:LO, :],
                        ilo_sb[:, t0 * 8:(t0 + n) * 8],
                        n * TILE, n * TILE, XLP, single_packet=False,
                        queue_num=0)
                    lo_bufs[ci] = b

                def emit_hi(ci):
                    t0, n = hi_calls[ci]
                    b = ghip.tile([128, GCALL, XLP], BF16, tag="ghi")
                    nc.gpsimd.dma_gather(
                        b[:, 0:n, :], xl_full[LO:NPAD, :],
                        ihi_sb[:, t0 * 8:(t0 + n) * 8],
                        n * TILE, n * TILE, XLP, single_packet=False,
                        queue_num=1)
                    hi_bufs[ci] = b

                next_lo = [0]
                next_hi = [0]

                def get_chunk_view(stream_t, nb):
                    # [128, nb, FDIM] view of nb consecutive stream tiles
                    if stream_t < T_lo:
                        ci, off = divmod(stream_t, GCALL)
                        while next_lo[0] <= ci:
                            emit_lo(next_lo[0])
                            next_lo[0] += 1
                        return lo_bufs[ci][:, off:off + nb, 0:FDIM]
                    st = stream_t - T_lo
                    ci, off = divmod(st, GCALL)
                    while next_hi[0] <= ci:
                        emit_hi(next_hi[0])
                        next_hi[0] += 1
                    return hi_bufs[ci][:, off:off + nb, 0:FDIM]

                h_next = hnp.tile([128, W * HLD], F32, tag="hnext")

                for w in range(W if nwin is None else nwin):
                    acc = psaccp.tile([128, 256], F32, tag="acc")
                    xr_win = xr_sb[:, w * FDIM:(w + 1) * FDIM]
                    # chunk this window's tiles: consecutive stream tiles,
                    # same phase block, same gather call, <= 4 tiles
                    chunks = []
                    for base, cnt in ((lo_start[w], ntiles[w, 0]),
                                      (hi_start[w], ntiles[w, 1])):
                        t = base
                        while t < base + cnt:
                            pos = t if t < T_lo else t - T_lo
                            nb = min(base + cnt - t, 4,
                                     GCALL - pos % GCALL)
                            chunks.append((int(t), int(nb)))
                            t += nb
                    nch = len(chunks)
                    for ci_, (b0, nb) in enumerate(chunks):
                        xl_q = get_chunk_view(b0, nb)
                        # dst row for these edges -> [1, nb*128] bf16
                        dstf = dstfp.tile([1, 4 * TILE], BF16, tag="dstf")
                        nc.sync.dma_start(
                            dstf[:, 0:nb * TILE],
                            dstf_d[0:1, b0 * TILE:(b0 + nb) * TILE])
                        # broadcast dst down partitions via K=1 matmul
                        psdb = psp.tile([128, 512], F32, tag="ps")
                        nc.tensor.matmul(
                            psdb[:, 0:nb * TILE], lhsT=ones1_sb[:],
                            rhs=dstf[:, 0:nb * TILE], start=True, stop=True)
                        # ST[j, e] = relu(1 - (dst_e - j)^2)  (exact onehot)
                        sq_t = edgep.tile([128, 512], F32, tag="sq")
                        nc.scalar.activation(
                            sq_t[:, 0:nb * TILE], psdb[:, 0:nb * TILE],
                            mybir.ActivationFunctionType.Square,
                            bias=consts["iota_neg"][:, 0:1])
                        ST_q = edgep.tile([128, 4, TILE], BF16, tag="ST")
                        nc.scalar.activation(
                            ST_q[:, 0:nb, :], sq_t[:, 0:nb * TILE],
                            mybir.ActivationFunctionType.Relu,
                            bias=1.0, scale=-1.0)
                        # s = xr_exp + xl accumulated in PSUM (two matmuls)
                        squad = pssqp.tile([128, 4, 256], F32, tag="squad")
                        for t in range(nb):
                            nc.tensor.matmul(
                                squad[:, t, 0:FDIM], lhsT=ST_q[:, t, :],
                                rhs=xr_win, start=True, stop=False)
                            nc.tensor.matmul(
                                squad[:, t, 0:FDIM], lhsT=consts["identb"][:],
                                rhs=xl_q[:, t, :], start=False, stop=True)
                        # m = leaky_relu(s) -> bf16 scratch in wm_q[:, :, 0:192]
                        wm_q = edgep.tile([128, 4, 256], BF16, tag="wm")
                        nc.scalar.activation(
                            wm_q[:, 0:nb, 0:FDIM], squad[:, 0:nb, 0:FDIM],
                            mybir.ActivationFunctionType.Prelu, alpha=SLOPE)
                        # em = m * att (bf16 2x mode, in place)
                        nc.vector.tensor_tensor(
                            out=wm_q[:, 0:nb, 0:FDIM], in0=wm_q[:, 0:nb, 0:FDIM],
                            in1=att[:].rearrange("p (o f) -> p o f", o=1)
                                .broadcast_to([128, nb, FDIM]),
                            op=mybir.AluOpType.mult)
                        # e (logits) then p = exp(e): f32 for the wm mult,
                        # bf16 straight into the scatter den columns
                        pe_q = edgep.tile([128, 4, 8], F32, tag="pe")
                        nc.vector.tensor_reduce(
                            out=pe_q[:, 0:nb, 4:7],
                            in_=wm_q[:, 0:nb, 0:FDIM].rearrange(
                                "p t (h c) -> p t h c", h=H),
                            axis=mybir.AxisListType.X, op=mybir.AluOpType.add)
                        nc.scalar.activation(
                            wm_q[:, 0:nb, FDIM:FDIM + H], pe_q[:, 0:nb, 4:7],
                            mybir.ActivationFunctionType.Exp)
                        # wm = p * xl (overwrites em scratch; all bf16)
                        nc.vector.tensor_tensor(
                            out=wm_q[:, 0:nb, 0:FDIM].rearrange(
                                "p t (h c) -> p t h c", h=H),
                            in0=xl_q.rearrange("p t (h c) -> p t h c", h=H),
                            in1=wm_q[:, 0:nb, FDIM:FDIM + H].broadcast_to(
                                [128, nb, H, HLD]),
                            op=mybir.AluOpType.mult)
                        # S onehot (edge-partition layout), bf16
                        S_q = edgep.tile([128, 4, TILE], BF16, tag="Sq")
                        nc.vector.tensor_tensor(
                            out=S_q[:, 0:nb, :],
                            in0=consts["iota_row"][:].rearrange(
                                "p (o f) -> p o f", o=1)
                                .broadcast_to([128, nb, TILE]),
                            in1=dstw_sb[:, b0:b0 + nb].broadcast_to(
                                [128, nb, TILE]),
                            op=mybir.AluOpType.is_equal)
                        for t in range(nb):
                            nc.tensor.matmul(
                                acc[:, 0:ACC_COLS], lhsT=S_q[:, t, :],
                                rhs=wm_q[:, t, 0:ACC_COLS],
                                start=(ci_ == 0 and t == 0),
                                stop=(ci_ == nch - 1 and t == nb - 1))

                    # ---- finalize window ----
                    accs = acc
                    fin = finp.tile([128, 8], F32, tag="fin")
                    nc.vector.tensor_scalar(
                        out=fin[:, 0:3], in0=accs[:, 192:195],
                        scalar1=3.0, scalar2=1e-16,
                        op0=mybir.AluOpType.mult, op1=mybir.AluOpType.add)
                    nc.vector.reciprocal(fin[:, 4:7], fin[:, 0:3])
                    u_t = finp.tile([128, FDIM], F32, tag="u")
                    for h in range(H):
                        nc.vector.tensor_scalar(
                            out=u_t[:, h * HLD:(h + 1) * HLD],
                            in0=accs[:, h * HLD:(h + 1) * HLD],
                            scalar1=fin[:, 4 + h:5 + h], scalar2=None,
                            op0=mybir.AluOpType.mult)
                    v_t = finp.tile([128, HLD], F32, tag="v")
                    nc.vector.tensor_tensor(
                        out=v_t[:], in0=u_t[:, 0:HLD], in1=u_t[:, HLD:2 * HLD],
                        op=mybir.AluOpType.add)
                    v2_t = finp.tile([128, HLD], F32, tag="v2")
                    nc.vector.tensor_tensor(
                        out=v2_t[:], in0=v_t[:], in1=u_t[:, 2 * HLD:3 * HLD],
                        op=mybir.AluOpType.add)
                    v3_t = finp.tile([128, HLD], F32, tag="v3")
                    nc.vector.tensor_tensor(
                        out=v3_t[:], in0=v2_t[:], in1=gbias[:],
                        op=mybir.AluOpType.add)
                    nc.scalar.activation(
                        h_next[:, w * HLD:(w + 1) * HLD], v3_t[:],
                        mybir.ActivationFunctionType.Relu)

                # ---- transpose h_next -> hT for next phase ----
                hT = htp.tile([HLD + 1, NPB], F32, tag="hT")
                nc.vector.memset(hT[HLD:HLD + 1, :], 1.0)
                for t in range(W):
                    pst = psp.tile([HLD, 128], F32, tag="ps")
                    nc.tensor.transpose(
                        pst[:], in_=h_next[:, t * HLD:(t + 1) * HLD],
                        identity=consts["ident"][:])
                    nc.scalar.activation(
                        hT[0:HLD, t * 128:(t + 1) * 128], pst[:],
                        mybir.ActivationFunctionType.Copy)

            # ---- decoder ----
            y0T = bigp.tile([HLD, NPB], F32, tag="big")
            for t in range(W):
                ps = psp.tile([HLD, 128], F32, tag="ps")
                nc.tensor.matmul(ps[:], lhsT=consts["dec_w0"][:],
                                 rhs=hT[0:HLD, t * 128:(t + 1) * 128],
                                 start=True, stop=True)
                nc.scalar.activation(y0T[:, t * 128:(t + 1) * 128], ps[:],
                                     mybir.ActivationFunctionType.Relu,
                                     bias=consts["dec_b0"][:, 0:1])
            outT_sb = bigp.tile([ODIM, NPB], F32, tag="big")
            for t in range(W):
                ps = psp.tile([ODIM, 128], F32, tag="ps")
                nc.tensor.matmul(ps[:], lhsT=consts["dec_w1"][:],
                                 rhs=y0T[:, t * 128:(t + 1) * 128],
                                 start=True, stop=True)
                nc.scalar.activation(outT_sb[:, t * 128:(t + 1) * 128], ps[:],
                                     mybir.ActivationFunctionType.Relu,
                                     bias=consts["dec_b1"][:, 0:1])
            nc.sync.dma_start(out_d[:], outT_sb[:])

    nc.compile()
    return nc


# ----------------------------------------------------------------------------
# host orchestration
# ----------------------------------------------------------------------------

def make_in_maps(inputs, plan):
    x = np.asarray(inputs["x"], np.float32)
    xpad = np.zeros((NPAD, IDIM), np.float32)
    xpad[:N] = x

    def col(b):
        return np.ascontiguousarray(np.asarray(b, np.float32).reshape(-1, 1))

    iota_row = np.tile(np.arange(128, dtype=np.float32), (128, 1)).astype(
        __import__("ml_dtypes").bfloat16)
    ident = np.eye(128, dtype=np.float32)

    def wplus(wname, bname):
        wm = np.asarray(inputs[wname], np.float32)
        bm = np.asarray(inputs[bname], np.float32)
        return np.ascontiguousarray(np.vstack([wm, bm[None, :]]))

    shared = {
        "enc_w0": np.ascontiguousarray(np.asarray(inputs["enc_w0"], np.float32)),
        "enc_b0": col(inputs["enc_b0"]),
        "enc_w1": np.ascontiguousarray(np.asarray(inputs["enc_w1"], np.float32)),
        "enc_b1": col(inputs["enc_b1"]),
        "dec_w0": np.ascontiguousarray(np.asarray(inputs["dec_w0"], np.float32)),
        "dec_b0": col(inputs["dec_b0"]),
        "dec_w1": np.ascontiguousarray(np.asarray(inputs["dec_w1"], np.float32)),
        "dec_b1": col(inputs["dec_b1"]),
        "wl0": wplus("gat0_wl", "gat0_bl"),
        "wr0": wplus("gat0_wr", "gat0_br"),
        "wl1": wplus("gat1_wl", "gat1_bl"),
        "wr1": wplus("gat1_wr", "gat1_br"),
        "att0": np.tile(np.asarray(inputs["gat0_att"], np.float32)
                        .reshape(1, FDIM), (128, 1)).astype(
                        __import__("ml_dtypes").bfloat16),
        "att1": np.tile(np.asarray(inputs["gat1_att"], np.float32)
                        .reshape(1, FDIM), (128, 1)).astype(
                        __import__("ml_dtypes").bfloat16),
        "gbias0": np.tile(np.asarray(inputs["gat0_bias"], np.float32)
                          .reshape(1, HLD), (128, 1)),
        "gbias1": np.tile(np.asarray(inputs["gat1_bias"], np.float32)
                          .reshape(1, HLD), (128, 1)),
        "iota_row": iota_row,
        "ident": ident,
        "iota_neg": -np.arange(128, dtype=np.float32).reshape(128, 1),
        "identb": np.eye(128, dtype=np.float32).astype(
            __import__("ml_dtypes").bfloat16),
        "ones1": np.ones((1, 128), np.float16).astype(np.float32).astype(
            __import__("ml_dtypes").bfloat16).reshape(1, 128),
    }
    maps = []
    for d in range(NCORES):
        m = dict(shared)
        m["xT"] = np.ascontiguousarray(xpad[d * NPB:(d + 1) * NPB].T)
        m["idx_lo"] = plan["idx_lo_w"][d]
        m["idx_hi"] = plan["idx_hi_w"][d]
        m["idx_xr"] = plan["idx_xr_w"][d]
        m["dstw"] = plan["dstw_w"][d].astype(
            __import__("ml_dtypes").bfloat16)
        m["dstf"] = np.ascontiguousarray(
            plan["dstw_w"][d].T.reshape(1, -1)).astype(
            __import__("ml_dtypes").bfloat16)
        maps.append(m)
    return maps


def kernel(**inputs):
    from concourse.bass_utils import run_bass_kernel_spmd

    plan = build_edge_plan(np.asarray(inputs["edgeIdx"]))
    nc = build_nc(plan)
    in_maps = make_in_maps(inputs, plan)
    res = run_bass_kernel_spmd(nc, in_maps, list(range(NCORES)))
    outs = res.results
    full = np.concatenate([outs[d]["outT"].T for d in range(NCORES)], 0)
    return np.ascontiguousarray(full[:N]).astype(np.float32)
